# revision 10
# baseline (speedup 1.0000x reference)
"""Multi-head self-attention Trainium2 kernel (8-core SPMD).

Problem: B=4, S=2048, E=1024, 16 heads x 64 dim, int mask, softmax attention.

Sharding: core c handles batch b=c//2 and head-half hh=c%2 (8 heads).
Each core computes Yp = Attn(X[b])[:, heads(hh)] @ wO[rows(hh)]  -> [S, E]
partial product; host sums the two partials per batch and adds bO.

Per-core data path (all layouts chosen so no on-device transposes needed):
  phase 1: QT = (wQ/8).T @ X.T   [512,2048]  (d_all on partition, fp32r matmul)
           KT =  wK.T   @ X.T    [512,2048]
           V  =  X @ wV          [2048,512]  stored bf16 with a fused ones
                                             column per head ([V_h | 1] width 65)
  phase 2 (per head-pair p, per q-half): flash-style over 16 k-tiles:
           S^T[k,q] = K @ Q^T  (two heads row-packed on the PE, contraction 64)
           exp on ACT (PSUM->SBUF bf16), mask multiply on DVE,
           [V_h|1].T @ P^T accumulated in PSUM -> out^T rows 0..63, rowsum row 64.
           Rowsum is DMA-broadcast across partitions, reciprocal on DVE, and
           normalization is fused into the PSUM->SBUF evacuation multiply.
  phase 3: Y = (out^T).T @ wO  via lhsT = out^T tiles (fp32r).
"""

import sys

if "/opt/trn_rl_repo" not in sys.path:
    sys.path.insert(0, "/opt/trn_rl_repo")

import numpy as np
import ml_dtypes

import concourse.bass as bass
import concourse.tile as tile
from concourse import bacc, mybir
from concourse.bass_utils import run_bass_kernel_spmd

F32 = mybir.dt.float32
BF16 = mybir.dt.bfloat16
F32R = mybir.dt.float32r
AF = mybir.ActivationFunctionType

S = 2048      # sequence length
E = 1024      # embed dim
DH = 512      # d_all per core (8 heads x 64)
D = 64        # head dim
H = 8         # heads per core
NE = 8        # embed 128-tiles
ND = 4        # d_all 128-tiles (= head pairs)
NS = 16       # seq 128-tiles
NK = 16       # k 128-tiles
V1W = D + 1   # V columns per head incl. ones column


def _emit(nc, tc, ctx, d):
    P = 128
    glob = ctx.enter_context(tc.tile_pool(name="glob", bufs=1))

    qt = glob.tile([P, ND * S], BF16)    # QT: [r, p*2048+q], d_all = 128p+r
    kt = glob.tile([P, ND * S], BF16)
    v1 = glob.tile([P, NS * H * V1W], BF16)  # V1: [s%128, st*520 + h*65 + j]
    bq = glob.tile([P, ND], F32)
    bk = glob.tile([P, ND], F32)
    bvb = glob.tile([P, DH], F32)        # bV broadcast across partitions

    nc.sync.dma_start(bq[:], d["bQ"].ap().rearrange("(n p) -> p n", p=P))
    nc.sync.dma_start(bk[:], d["bK"].ap().rearrange("(n p) -> p n", p=P))
    nc.sync.dma_start(
        bvb[:], d["bV"].ap().rearrange("(a s) -> a s", a=1).partition_broadcast(P)
    )

    # ---------------- phase 1: projections ----------------
    with (
        tc.tile_pool(name="p1sb", bufs=1) as p1sb,
        tc.tile_pool(name="p1ps", bufs=4, space="PSUM") as p1ps,
    ):
        xt = p1sb.tile([P, NE * S], F32R)    # X^T: [r, e*2048+s], embed = 128e+r
        wq = p1sb.tile([P, NE * DH], F32R)   # wQ: [r, e*512+c]
        wk = p1sb.tile([P, NE * DH], F32R)
        wv = p1sb.tile([P, NE * DH], F32R)

        for nm, t in (("wQ", wq), ("wK", wk), ("wV", wv)):
            nc.sync.dma_start(
                t[:].rearrange("p (e c) -> p e c", c=DH),
                d[nm].ap().rearrange("(e p) c -> p e c", p=P),
            )
        for e in range(NE):
            nc.sync.dma_start(
                xt[:, e * S:(e + 1) * S],
                d["XT"].ap().rearrange("(e p) s -> e p s", p=P)[e],
            )

        # QT / KT passes: QT[c, s] = sum_e wQ[e, c] * XT[e, s]
        for sc in range(4):            # seq chunks of 512
            for w_sb, out_t, b_t in ((wq, qt, bq), (wk, kt, bk)):
                for dd in range(ND):   # d_all tile (pair)
                    ps = p1ps.tile([P, 512], F32, tag="proj")
                    for e in range(NE):
                        nc.tensor.matmul(
                            ps[:],
                            w_sb[:, e * DH + dd * P: e * DH + (dd + 1) * P],
                            xt[:, e * S + sc * 512: e * S + sc * 512 + 512],
                            start=(e == 0), stop=(e == NE - 1),
                        )
                    nc.scalar.activation(
                        out_t[:, dd * S + sc * 512: dd * S + sc * 512 + 512],
                        ps[:], AF.Identity, bias=b_t[:, dd:dd + 1],
                    )

        # ones columns of V1 (before V writes; disjoint columns)
        nc.vector.memset(
            v1[:].rearrange("p (t h j) -> p t h j", t=NS, j=V1W)[:, :, :, D:D + 1],
            1.0,
        )

        # V pass: V[s, c] = sum_e XT[e, s] * wV[e, c]
        for st in range(NS):
            ps = p1ps.tile([P, 512], F32, tag="proj")
            for e in range(NE):
                nc.tensor.matmul(
                    ps[:],
                    xt[:, e * S + st * P: e * S + (st + 1) * P],
                    wv[:, e * DH:(e + 1) * DH],
                    start=(e == 0), stop=(e == NE - 1),
                )
            dst = v1[:, st * H * V1W:(st + 1) * H * V1W].rearrange(
                "p (h j) -> p h j", j=V1W
            )[:, :, 0:D]
            nc.vector.tensor_add(
                dst,
                ps[:].rearrange("p (h j) -> p h j", j=D),
                bvb[:].rearrange("p (h j) -> p h j", j=D),
            )

    # ---------------- phases 2+3 ----------------
    with tc.tile_pool(name="p23sb", bufs=1) as p23sb:
        otn = p23sb.tile([P, ND * S], F32R)   # normalized out^T
        wo = p23sb.tile([P, ND * E], F32R)    # wO: [r, p*1024+c], d_all = 128p+r

        nc.sync.dma_start(
            wo[:].rearrange("p (n c) -> p n c", c=E),
            d["wO"].ap().rearrange("(n p) c -> p n c", p=P),
        )

        with (
            tc.tile_pool(name="p2sb", bufs=1) as p2sb,
            tc.tile_pool(name="p2sc", bufs=2, space="PSUM") as p2sc,
            tc.tile_pool(name="p2pv", bufs=2, space="PSUM") as p2pv,
            tc.tile_pool(name="p2str", bufs=2) as p2str,
            tc.tile_pool(name="p2nrm", bufs=1) as p2nrm,
            tc.tile_pool(name="p2dram", bufs=2, space="DRAM") as p2dram,
        ):
            mt = p2sb.tile([P, NK * S], BF16)  # mask^T: [r, k*2048+q]
            for k in range(NK):
                nc.sync.dma_start(
                    mt[:, k * S:(k + 1) * S],
                    d["maskT"].ap().rearrange("(k p) q -> k p q", p=P)[k],
                )

            for p in range(ND):          # head pair
                for qh in range(2):      # q half (1024)
                    pv1 = p2pv.tile([V1W, 1024], F32, tag="pv")
                    pv2 = p2pv.tile([V1W, 1024], F32, tag="pv")
                    qbase = p * S + qh * 1024
                    for k in range(NK):
                        s1 = p2sc.tile([P, 1024], F32, tag="sc")
                        s2 = p2sc.tile([P, 1024], F32, tag="sc")
                        for h, sps in ((0, s1), (1, s2)):
                            lo = h * D
                            hi = lo + D
                            for c in range(2):
                                nc.tensor.matmul(
                                    sps[:, c * 512:(c + 1) * 512],
                                    kt[lo:hi, p * S + k * P: p * S + (k + 1) * P],
                                    qt[lo:hi, qbase + c * 512: qbase + (c + 1) * 512],
                                    start=True, stop=True,
                                )
                        e1 = p2str.tile([P, 1024], BF16, tag="es")
                        e2 = p2str.tile([P, 1024], BF16, tag="es")
                        nc.scalar.activation(e1[:], s1[:], AF.Exp)
                        nc.scalar.activation(e2[:], s2[:], AF.Exp)
                        pr1 = p2str.tile([P, 1024], BF16, tag="pr")
                        pr2 = p2str.tile([P, 1024], BF16, tag="pr")
                        mv = mt[:, k * S + qh * 1024: k * S + qh * 1024 + 1024]
                        nc.vector.tensor_mul(pr1[:], e1[:], mv)
                        nc.vector.tensor_mul(pr2[:], e2[:], mv)
                        for h, pv, pr in ((0, pv1, pr1), (1, pv2, pr2)):
                            head = 2 * p + h
                            for c in range(2):
                                nc.tensor.matmul(
                                    pv[:, c * 512:(c + 1) * 512],
                                    v1[:, k * H * V1W + head * V1W:
                                          k * H * V1W + head * V1W + V1W],
                                    pr[:, c * 512:(c + 1) * 512],
                                    start=(k == 0), stop=(k == NK - 1),
                                )
                    # stage PV out of PSUM, then normalize via a DRAM
                    # round-trip broadcast of the reciprocal rowsums.
                    # All DVE ops keep equal SBUF base partitions.
                    st = p2nrm.tile([P, 1024], F32, tag="st")
                    nc.vector.tensor_copy(st[0:D, :], pv1[0:D, :])
                    nc.vector.tensor_copy(st[D:P, :], pv2[0:D, :])
                    rs = p2nrm.tile([P, 2 * 1024], F32, tag="rs")
                    nc.vector.reciprocal(rs[D:D + 1, 0:1024], pv1[D:D + 1, :])
                    nc.vector.reciprocal(rs[D:D + 1, 1024:2048], pv2[D:D + 1, :])
                    dsc1 = p2dram.tile([1, 1024], F32, tag="d1")
                    dsc2 = p2dram.tile([1, 1024], F32, tag="d2")
                    nc.sync.dma_start(dsc1[:], rs[D:D + 1, 0:1024])
                    nc.sync.dma_start(dsc2[:], rs[D:D + 1, 1024:2048])
                    rb = p2nrm.tile([P, 1024], F32, tag="rb")
                    nc.sync.dma_start(rb[0:D, :], dsc1[:].partition_broadcast(D))
                    nc.sync.dma_start(rb[D:P, :], dsc2[:].partition_broadcast(D))
                    nc.vector.tensor_mul(
                        otn[0:D, qbase:qbase + 1024], st[0:D, :], rb[0:D, :]
                    )
                    nc.vector.tensor_mul(
                        otn[D:P, qbase:qbase + 1024], st[D:P, :], rb[D:P, :]
                    )

        # phase 3: Y = out.T @ wO   (lhsT = otn tiles)
        with (
            tc.tile_pool(name="p3sb", bufs=4) as p3sb,
            tc.tile_pool(name="p3ps", bufs=4, space="PSUM") as p3ps,
        ):
            for qi in range(NS):
                for ec in range(2):
                    yps = p3ps.tile([P, 512], F32, tag="y")
                    for p in range(ND):
                        nc.tensor.matmul(
                            yps[:],
                            otn[:, p * S + qi * P: p * S + (qi + 1) * P],
                            wo[:, p * E + ec * 512: p * E + ec * 512 + 512],
                            start=(p == 0), stop=(p == ND - 1),
                        )
                    ysb = p3sb.tile([P, 512], F32, tag="ys")
                    nc.scalar.copy(ysb[:], yps[:])
                    nc.sync.dma_start(
                        d["Yp"].ap()[qi * P:(qi + 1) * P, ec * 512:(ec + 1) * 512],
                        ysb[:],
                    )


def build_module():
    from contextlib import ExitStack

    nc = bacc.Bacc("TRN2", target_bir_lowering=False, debug=False)
    d = {
        "XT": nc.dram_tensor("XT", [E, S], F32R, kind="ExternalInput"),
        "maskT": nc.dram_tensor("maskT", [S, S], BF16, kind="ExternalInput"),
        "wQ": nc.dram_tensor("wQ", [E, DH], F32R, kind="ExternalInput"),
        "wK": nc.dram_tensor("wK", [E, DH], F32R, kind="ExternalInput"),
        "wV": nc.dram_tensor("wV", [E, DH], F32R, kind="ExternalInput"),
        "wO": nc.dram_tensor("wO", [DH, E], F32R, kind="ExternalInput"),
        "bQ": nc.dram_tensor("bQ", [DH], F32, kind="ExternalInput"),
        "bK": nc.dram_tensor("bK", [DH], F32, kind="ExternalInput"),
        "bV": nc.dram_tensor("bV", [DH], F32, kind="ExternalInput"),
        "Yp": nc.dram_tensor("Yp", [S, E], F32, kind="ExternalOutput"),
    }
    with tile.TileContext(nc) as tc:
        with ExitStack() as ctx:
            _emit(nc, tc, ctx, d)
    nc.compile()
    return nc


def make_in_maps(X, mask, wQ, bQ, wK, bK, wV, bV, wO, bO):
    """Per-core input dicts. Core c: batch c//2, head-half c%2."""
    in_maps = []
    for c in range(8):
        b, hh = c // 2, c % 2
        cols = slice(hh * DH, (hh + 1) * DH)
        in_maps.append({
            "XT": np.ascontiguousarray(np.asarray(X[b]).T),
            "maskT": np.ascontiguousarray(
                np.asarray(mask[b, 0]).T
            ).astype(ml_dtypes.bfloat16),
            "wQ": np.ascontiguousarray(np.asarray(wQ[:, cols])) * np.float32(0.125),
            "wK": np.ascontiguousarray(np.asarray(wK[:, cols])),
            "wV": np.ascontiguousarray(np.asarray(wV[:, cols])),
            "wO": np.ascontiguousarray(np.asarray(wO[cols, :])),
            "bQ": np.ascontiguousarray(np.asarray(bQ[cols])) * np.float32(0.125),
            "bK": np.ascontiguousarray(np.asarray(bK[cols])),
            "bV": np.ascontiguousarray(np.asarray(bV[cols])),
        })
    return in_maps


_NC = None


def kernel(X, mask, wQ, bQ, wK, bK, wV, bV, wO, bO):
    global _NC
    if _NC is None:
        _NC = build_module()
    in_maps = make_in_maps(X, mask, wQ, bQ, wK, bK, wV, bV, wO, bO)
    res = run_bass_kernel_spmd(_NC, in_maps, list(range(8)))
    B = 4
    Y = np.empty((B, S, E), dtype=np.float32)
    bO = np.asarray(bO, dtype=np.float32)
    for b in range(B):
        Y[b] = res.results[2 * b]["Yp"] + res.results[2 * b + 1]["Yp"] + bO
    return Y


# revision 18
# speedup vs baseline: 158.4087x; 158.4087x over previous
"""Multi-head self-attention Trainium2 kernel (8-core SPMD).

Problem: B=4, S=2048, E=1024, 16 heads x 64 dim, int mask, softmax attention.

Sharding: core c handles batch b=c//2 and head-half hh=c%2 (8 heads).
Each core computes Yp = Attn(X[b])[:, heads(hh)] @ wO[rows(hh)]  -> [S, E]
partial product; host sums the two partials per batch and adds bO.

Per-core data path (all layouts chosen so no on-device transposes needed):
  phase 1: QT = (wQ/8).T @ X.T   [512,2048]  (d_all on partition, fp32r matmul)
           KT =  wK.T   @ X.T    [512,2048]
           V  =  X @ wV          [2048,512]  stored bf16 with a fused ones
                                             column per head ([V_h | 1] width 65)
  phase 2 (per head-pair p, per q-half): flash-style over 16 k-tiles:
           S^T[k,q] = K @ Q^T  (two heads row-packed on the PE, contraction 64)
           exp on ACT (PSUM->SBUF bf16), mask multiply on DVE,
           [V_h|1].T @ P^T accumulated in PSUM -> out^T rows 0..63, rowsum row 64.
           Rowsum is DMA-broadcast across partitions, reciprocal on DVE, and
           normalization is fused into the PSUM->SBUF evacuation multiply.
  phase 3: Y = (out^T).T @ wO  via lhsT = out^T tiles (fp32r).
"""

import sys

if "/opt/trn_rl_repo" not in sys.path:
    sys.path.insert(0, "/opt/trn_rl_repo")

import numpy as np
import ml_dtypes

import concourse.bass as bass
import concourse.tile as tile
from concourse import bacc, mybir
from concourse.bass_utils import run_bass_kernel_spmd

F32 = mybir.dt.float32
BF16 = mybir.dt.bfloat16
F32R = mybir.dt.float32r
AF = mybir.ActivationFunctionType

S = 2048      # sequence length
E = 1024      # embed dim
DH = 512      # d_all per core (8 heads x 64)
D = 64        # head dim
H = 8         # heads per core
NE = 8        # embed 128-tiles
ND = 4        # d_all 128-tiles (= head pairs)
NS = 16       # seq 128-tiles
NK = 16       # k 128-tiles
V1W = D + 1   # V columns per head incl. ones column


def _emit(nc, tc, ctx, d):
    P = 128
    glob = ctx.enter_context(tc.tile_pool(name="glob", bufs=1))

    qt = glob.tile([P, ND * S], BF16)    # QT: [r, p*2048+q], d_all = 128p+r
    kt = glob.tile([P, ND * S], BF16)
    v1 = glob.tile([P, NS * H * V1W], BF16)  # V1: [s%128, st*520 + h*65 + j]
    mt = glob.tile([P, NK * S], BF16)    # mask^T: [r, k*2048+q]
    bq = glob.tile([P, ND], F32)
    bk = glob.tile([P, ND], F32)
    bvb = glob.tile([P, DH], F32)        # bV broadcast across partitions

    # PSUM pools shared by phase 1 and phase 2 so attention work can begin
    # while projections are still draining:
    #   "sc" (banks 0-3): QT/KT projection groups, then QK score tiles
    #   "pv" (banks 4-7): V projection groups, then PV accumulators
    ps_sc = ctx.enter_context(tc.tile_pool(name="ps_sc", bufs=2, space="PSUM"))
    ps_pv = ctx.enter_context(tc.tile_pool(name="ps_pv", bufs=2, space="PSUM"))

    nc.sync.dma_start(bq[:], d["bQ"].ap().rearrange("(n p) -> p n", p=P))
    nc.sync.dma_start(bk[:], d["bK"].ap().rearrange("(n p) -> p n", p=P))
    nc.sync.dma_start(
        bvb[:], d["bV"].ap().rearrange("(a s) -> a s", a=1).partition_broadcast(P)
    )

    # ---------------- phase 1: projections ----------------
    with tc.tile_pool(name="p1sb", bufs=1) as p1sb:
        xt = p1sb.tile([P, NE * S], BF16)    # X^T: [r, e*2048+s], embed = 128e+r
        wq = p1sb.tile([P, NE * DH], BF16)   # wQ: [r, e*512+c]
        wk = p1sb.tile([P, NE * DH], BF16)
        wv = p1sb.tile([P, NE * DH], BF16)

        for nm, t in (("wQ", wq), ("wK", wk), ("wV", wv)):
            nc.sync.dma_start(
                t[:].rearrange("p (e c) -> p e c", c=DH),
                d[nm].ap().rearrange("(e p) c -> p e c", p=P),
            )
        for e in range(NE):
            nc.sync.dma_start(
                xt[:, e * S:(e + 1) * S],
                d["XT"].ap().rearrange("(e p) s -> e p s", p=P)[e],
            )
        for k in range(NK):
            nc.sync.dma_start(
                mt[:, k * S:(k + 1) * S],
                d["maskT"].ap().rearrange("(k p) q -> k p q", p=P)[k],
            )

        # QT / KT passes: QT[c, s] = sum_e wQ[e, c] * XT[e, s]
        # dd (head pair) outermost so pair 0 finishes first and phase-2
        # score matmuls can start while later pairs still project.
        for dd in range(ND):           # d_all tile (pair)
            for w_sb, out_t, b_t in ((wk, kt, bk), (wq, qt, bq)):
                for sc in range(4):    # seq chunks of 512
                    ps = ps_sc.tile([P, 512], F32, tag="sc")
                    for e in range(NE):
                        nc.tensor.matmul(
                            ps[:],
                            w_sb[:, e * DH + dd * P: e * DH + (dd + 1) * P],
                            xt[:, e * S + sc * 512: e * S + sc * 512 + 512],
                            start=(e == 0), stop=(e == NE - 1),
                        )
                    nc.scalar.activation(
                        out_t[:, dd * S + sc * 512: dd * S + sc * 512 + 512],
                        ps[:], AF.Identity, bias=b_t[:, dd:dd + 1],
                    )

        # ones columns of V1 (before V writes; disjoint columns)
        nc.vector.memset(
            v1[:].rearrange("p (t h j) -> p t h j", t=NS, j=V1W)[:, :, :, D:D + 1],
            1.0,
        )

        # V pass: V[s, c] = sum_e XT[e, s] * wV[e, c]
        for st in range(NS):
            ps = ps_pv.tile([P, 512], F32, tag="pv")
            for e in range(NE):
                nc.tensor.matmul(
                    ps[:],
                    xt[:, e * S + st * P: e * S + (st + 1) * P],
                    wv[:, e * DH:(e + 1) * DH],
                    start=(e == 0), stop=(e == NE - 1),
                )
            dst = v1[:, st * H * V1W:(st + 1) * H * V1W].rearrange(
                "p (h j) -> p h j", j=V1W
            )[:, :, 0:D]
            nc.vector.tensor_add(
                dst,
                ps[:].rearrange("p (h j) -> p h j", j=D),
                bvb[:].rearrange("p (h j) -> p h j", j=D),
            )

    # ---------------- phases 2+3 ----------------
    with tc.tile_pool(name="p23sb", bufs=1) as p23sb:
        otn = p23sb.tile([P, ND * S], BF16)   # normalized out^T
        wo = p23sb.tile([P, ND * E], BF16)    # wO: [r, p*1024+c], d_all = 128p+r

        nc.sync.dma_start(
            wo[:].rearrange("p (n c) -> p n c", c=E),
            d["wO"].ap().rearrange("(n p) c -> p n c", p=P),
        )

        with (
            tc.tile_pool(name="p2str", bufs=6) as p2str,
            tc.tile_pool(name="p2nrm", bufs=1) as p2nrm,
            tc.tile_pool(name="p2dram", bufs=2, space="DRAM") as p2dram,
            tc.tile_pool(name="p3sb", bufs=4) as p3sb,
        ):
            def emit_y_half(qh):
                # Y = out.T @ wO for q tiles of this half (lhsT = otn tiles)
                for qi in range(qh * NS // 2, (qh + 1) * NS // 2):
                    for ec in range(2):
                        yps = ps_sc.tile([P, 512], F32, tag="sc")
                        for p in range(ND):
                            nc.tensor.matmul(
                                yps[:],
                                otn[:, p * S + qi * P: p * S + (qi + 1) * P],
                                wo[:, p * E + ec * 512: p * E + ec * 512 + 512],
                                start=(p == 0), stop=(p == ND - 1),
                            )
                        ysb = p3sb.tile([P, 512], F32, tag="ys")
                        nc.scalar.copy(ysb[:], yps[:])
                        nc.sync.dma_start(
                            d["Yp"].ap()[qi * P:(qi + 1) * P,
                                         ec * 512:(ec + 1) * 512],
                            ysb[:],
                        )

            for qh in range(2):          # q half (1024)
              for p in range(ND):          # head pair
                if True:
                    pv1 = ps_pv.tile([V1W, 1024], F32, tag="pv")
                    pv2 = ps_pv.tile([V1W, 1024], F32, tag="pv")
                    qbase = p * S + qh * 1024
                    for k in range(NK):
                        s1 = ps_sc.tile([P, 1024], F32, tag="sc")
                        s2 = ps_sc.tile([P, 1024], F32, tag="sc")
                        for h, sps in ((0, s1), (1, s2)):
                            lo = h * D
                            hi = lo + D
                            for c in range(2):
                                nc.tensor.matmul(
                                    sps[:, c * 512:(c + 1) * 512],
                                    kt[lo:hi, p * S + k * P: p * S + (k + 1) * P],
                                    qt[lo:hi, qbase + c * 512: qbase + (c + 1) * 512],
                                    start=True, stop=True,
                                )
                        e1 = p2str.tile([P, 1024], BF16, tag="es")
                        e2 = p2str.tile([P, 1024], BF16, tag="es")
                        nc.scalar.activation(e1[:], s1[:], AF.Exp)
                        nc.scalar.activation(e2[:], s2[:], AF.Exp)
                        pr1 = p2str.tile([P, 1024], BF16, tag="pr")
                        pr2 = p2str.tile([P, 1024], BF16, tag="pr")
                        mv = mt[:, k * S + qh * 1024: k * S + qh * 1024 + 1024]
                        nc.vector.tensor_mul(pr1[:], e1[:], mv)
                        nc.vector.tensor_mul(pr2[:], e2[:], mv)
                        for h, pv, pr in ((0, pv1, pr1), (1, pv2, pr2)):
                            head = 2 * p + h
                            for c in range(2):
                                nc.tensor.matmul(
                                    pv[:, c * 512:(c + 1) * 512],
                                    v1[:, k * H * V1W + head * V1W:
                                          k * H * V1W + head * V1W + V1W],
                                    pr[:, c * 512:(c + 1) * 512],
                                    start=(k == 0), stop=(k == NK - 1),
                                )
                    # stage PV out of PSUM, then normalize via a DRAM
                    # round-trip broadcast of the reciprocal rowsums.
                    # All DVE ops keep equal SBUF base partitions.
                    st = p2nrm.tile([P, 1024], F32, tag="st")
                    nc.vector.tensor_copy(st[0:D, :], pv1[0:D, :])
                    nc.vector.tensor_copy(st[D:P, :], pv2[0:D, :])
                    rs = p2nrm.tile([P, 2 * 1024], F32, tag="rs")
                    nc.vector.reciprocal(rs[D:D + 1, 0:1024], pv1[D:D + 1, :])
                    nc.vector.reciprocal(rs[D:D + 1, 1024:2048], pv2[D:D + 1, :])
                    dsc1 = p2dram.tile([1, 1024], F32, tag="d1")
                    dsc2 = p2dram.tile([1, 1024], F32, tag="d2")
                    nc.sync.dma_start(dsc1[:], rs[D:D + 1, 0:1024])
                    nc.sync.dma_start(dsc2[:], rs[D:D + 1, 1024:2048])
                    rb = p2nrm.tile([P, 1024], F32, tag="rb")
                    nc.sync.dma_start(rb[0:D, :], dsc1[:].partition_broadcast(D))
                    nc.sync.dma_start(rb[D:P, :], dsc2[:].partition_broadcast(D))
                    nc.vector.tensor_mul(
                        otn[0:D, qbase:qbase + 1024], st[0:D, :], rb[0:D, :]
                    )
                    nc.vector.tensor_mul(
                        otn[D:P, qbase:qbase + 1024], st[D:P, :], rb[D:P, :]
                    )
            emit_y_half(0)
            emit_y_half(1)



def build_module(reps=1):
    from contextlib import ExitStack

    nc = bacc.Bacc("TRN2", target_bir_lowering=False, debug=False)
    d = {
        "XT": nc.dram_tensor("XT", [E, S], BF16, kind="ExternalInput"),
        "maskT": nc.dram_tensor("maskT", [S, S], BF16, kind="ExternalInput"),
        "wQ": nc.dram_tensor("wQ", [E, DH], BF16, kind="ExternalInput"),
        "wK": nc.dram_tensor("wK", [E, DH], BF16, kind="ExternalInput"),
        "wV": nc.dram_tensor("wV", [E, DH], BF16, kind="ExternalInput"),
        "wO": nc.dram_tensor("wO", [DH, E], BF16, kind="ExternalInput"),
        "bQ": nc.dram_tensor("bQ", [DH], F32, kind="ExternalInput"),
        "bK": nc.dram_tensor("bK", [DH], F32, kind="ExternalInput"),
        "bV": nc.dram_tensor("bV", [DH], F32, kind="ExternalInput"),
        "Yp": nc.dram_tensor("Yp", [S, E], F32, kind="ExternalOutput"),
    }
    with tile.TileContext(nc) as tc:
        for _ in range(reps):
            with ExitStack() as ctx:
                _emit(nc, tc, ctx, d)
    nc.compile()
    return nc


def make_in_maps(X, mask, wQ, bQ, wK, bK, wV, bV, wO, bO):
    """Per-core input dicts. Core c: batch c//2, head-half c%2."""
    in_maps = []
    for c in range(8):
        b, hh = c // 2, c % 2
        cols = slice(hh * DH, (hh + 1) * DH)
        in_maps.append({
            "XT": np.ascontiguousarray(np.asarray(X[b]).T).astype(ml_dtypes.bfloat16),
            "maskT": np.ascontiguousarray(
                np.asarray(mask[b, 0]).T
            ).astype(ml_dtypes.bfloat16),
            "wQ": (np.asarray(wQ[:, cols]) * np.float32(0.125)).astype(ml_dtypes.bfloat16),
            "wK": np.asarray(wK[:, cols]).astype(ml_dtypes.bfloat16),
            "wV": np.asarray(wV[:, cols]).astype(ml_dtypes.bfloat16),
            "wO": np.asarray(wO[cols, :]).astype(ml_dtypes.bfloat16),
            "bQ": np.ascontiguousarray(np.asarray(bQ[cols])) * np.float32(0.125),
            "bK": np.ascontiguousarray(np.asarray(bK[cols])),
            "bV": np.ascontiguousarray(np.asarray(bV[cols])),
        })
    return in_maps


_NC = None


def kernel(X, mask, wQ, bQ, wK, bK, wV, bV, wO, bO):
    global _NC
    if _NC is None:
        _NC = build_module()
    in_maps = make_in_maps(X, mask, wQ, bQ, wK, bK, wV, bV, wO, bO)
    res = run_bass_kernel_spmd(_NC, in_maps, list(range(8)))
    B = 4
    Y = np.empty((B, S, E), dtype=np.float32)
    bO = np.asarray(bO, dtype=np.float32)
    for b in range(B):
        Y[b] = res.results[2 * b]["Yp"] + res.results[2 * b + 1]["Yp"] + bO
    return Y
